# revision 9
# baseline (speedup 1.0000x reference)
"""AutoCorrelation (Autoformer) Bass kernel for Trainium2, 8 NeuronCores.

Inputs (full): queries/keys/values [4, 207, 96, 8, 64] f32, attn_mask scalar.
Outputs: tuple (V, corr), each [4, 207, 96, 8, 64] f32.

Strategy: flatten (B,N) -> 828 independent slices of shape [L=96, H*E=512],
sharded over 8 cores (pad to 832 = 8*104). Each slice is processed fully
on-chip in the natural [L, HE] layout:
  - rfft/irfft of the circular cross-correlation are DFT matmuls contracting
    over L (or freq) on the partition axis; the +/- recombination of the
    complex product is absorbed into the inverse-DFT matmul weights. The
    corr path stays fp32 so top-k ranks agree with the fp32 reference.
  - top-k=4 per channel via DVE max8/max_index on the transposed corr.
  - the circular gather runs in the frequency domain (bf16): a weighted
    one-hot of the delays is built with fused tensor_scalar (is_equal, mult)
    ops, transposed on the tensor engine, DFT'd with a replicated-DFT matmul
    (which also sums the 4 taps), multiplied with VF, and inverse-DFT'd.
"""

import math
import sys

sys.path.insert(0, "/opt/trn_rl_repo")

import ml_dtypes
import numpy as np

import concourse.bacc as bacc
import concourse.bass as bass
import concourse.mybir as mybir
from concourse import tile
from concourse.bass_utils import run_bass_kernel_spmd

B, N, L, H, E = 4, 207, 96, 8, 64
HE = H * E          # 512
F = L // 2 + 1      # 49
F2 = 2 * F          # 98
PW = 128            # padded freq partitions: Re 0..48, Im 64..112
IMG = 64
TOPK = 4            # int(log(96))
NCORES = 8
NSLICES = B * N                      # 828
S = math.ceil(NSLICES / NCORES)      # 104
NT = HE // 128                       # 4 channel tiles of 128

FP = mybir.dt.float32
BF = mybir.dt.bfloat16
U32 = mybir.dt.uint32
AF = mybir.ActivationFunctionType
ALU = mybir.AluOpType
BF_NP = ml_dtypes.bfloat16


def _consts():
    t = np.arange(L)[:, None]
    f = np.arange(F)[None, :]
    C = np.cos(2 * np.pi * f * t / L)
    Sm = np.sin(2 * np.pi * f * t / L)
    Z15 = np.zeros((L, 64 - F))
    Dq = np.concatenate([C, Z15, -Sm, Z15], axis=1).astype(np.float32)   # [96, 128]
    Dsw = np.concatenate([-Sm, Z15, C, Z15], axis=1).astype(np.float32)  # [96, 128]
    wf = np.full(F, 2.0)
    wf[0] = 1.0
    wf[F - 1] = 1.0
    tau = np.arange(L)[None, :]
    fc = np.arange(F)[:, None]
    IC = (wf[:, None] / L) * np.cos(2 * np.pi * fc * tau / L)   # [49, 96]
    ISn = (wf[:, None] / L) * np.sin(2 * np.pi * fc * tau / L)
    Z15r = np.zeros((64 - F, L))
    M1 = np.concatenate([IC, Z15r, IC, Z15r], axis=0).astype(np.float32)   # [128, 96]
    M2P = np.concatenate([ISn, Z15r, -ISn, Z15r], axis=0).astype(np.float32)
    M2G = (-M2P).astype(np.float32)
    DR = Dq[np.arange(3 * 128) % L, :].astype(np.float32)       # [384, 128]
    iota = np.tile(np.arange(L, dtype=np.float32), (128, 1))    # [128, 96]
    ident = np.eye(128, dtype=np.float32)
    f32 = dict(Dq=Dq, M1=M1, M2P=M2P, ident=ident)
    b16 = dict(Dqb=Dq, Dswb=Dsw, M1b=M1, M2Gb=M2G,
               DR0b=DR[0:128], DR1b=DR[128:256], DR2b=DR[256:384],
               iotab=iota, identb=ident)
    b16 = {k: v.astype(BF_NP) for k, v in b16.items()}
    return f32, b16


def _build_program(n_slices):
    nc = bacc.Bacc("TRN2", target_bir_lowering=False, debug=False,
                   num_devices=NCORES)
    qs = nc.dram_tensor("qs", [n_slices, L, HE], FP, kind="ExternalInput")
    ks = nc.dram_tensor("ks", [n_slices, L, HE], FP, kind="ExternalInput")
    vs = nc.dram_tensor("vs", [n_slices, L, HE], BF, kind="ExternalInput")
    cf32, cb16 = _consts()
    cdram = {}
    for k, v in cf32.items():
        cdram[k] = nc.dram_tensor(k, list(v.shape), FP, kind="ExternalInput")
    for k, v in cb16.items():
        cdram[k] = nc.dram_tensor(k, list(v.shape), BF, kind="ExternalInput")
    co = nc.dram_tensor("co", [n_slices, L, HE], FP, kind="ExternalOutput")
    vo = nc.dram_tensor("vo", [n_slices, L, HE], FP, kind="ExternalOutput")

    with tile.TileContext(nc) as tc:
        with (
            tc.tile_pool(name="const", bufs=1) as cpool,
            tc.tile_pool(name="io", bufs=3) as io,
            tc.tile_pool(name="work", bufs=2) as wk,
            tc.tile_pool(name="small", bufs=2) as sm,
            tc.tile_pool(name="ps98", bufs=4, space="PSUM") as ps98,
            tc.tile_pool(name="ps96", bufs=2, space="PSUM") as ps96,
            tc.tile_pool(name="ps128", bufs=2, space="PSUM") as ps128,
        ):
            cb = {}
            for k, v in cf32.items():
                cb[k] = cpool.tile(list(v.shape), FP, tag=k, name=k)
                nc.sync.dma_start(out=cb[k][:], in_=cdram[k].ap())
            for k, v in cb16.items():
                cb[k] = cpool.tile(list(v.shape), BF, tag=k, name=k)
                nc.sync.dma_start(out=cb[k][:], in_=cdram[k].ap())

            for s in range(n_slices):
                qt = io.tile([L, HE], FP, tag="qt", name="qt")
                kt = io.tile([L, HE], FP, tag="kt", name="kt")
                vt = io.tile([L, HE], BF, tag="vt", name="vt")
                nc.sync.dma_start(out=qt[:], in_=qs.ap()[s])
                nc.sync.dma_start(out=kt[:], in_=ks.ap()[s])
                nc.sync.dma_start(out=vt[:], in_=vs.ap()[s])

                # ---- forward DFTs of q, k (fp32) ----
                QFp = ps98.tile([PW, HE], FP, tag="f98", name="QFp")
                KFp = ps98.tile([PW, HE], FP, tag="f98", name="KFp")
                nc.tensor.matmul(QFp[:], cb["Dq"][:], qt[:])
                nc.tensor.matmul(KFp[:], cb["Dq"][:], kt[:])

                # ---- complex products; recombination folded into M1/M2P ----
                qfs = wk.tile([PW, HE], FP, tag="qfs", name="qfs")
                nc.scalar.copy(qfs[:], QFp[:])
                m1t = wk.tile([PW, HE], FP, tag="m1t", name="m1t")
                m2t = wk.tile([PW, HE], FP, tag="m2t", name="m2t")
                nc.vector.tensor_mul(m1t[:], qfs[:], KFp[:])
                nc.vector.tensor_mul(m2t[:IMG, :], qfs[:IMG, :], KFp[IMG:, :])
                nc.vector.tensor_mul(m2t[IMG:, :], qfs[IMG:, :], KFp[:IMG, :])

                # ---- inverse DFT -> corr [96, 512] fp32 ----
                corrp = ps96.tile([L, HE], FP, tag="f96", name="corrp")
                nc.tensor.matmul(corrp[:], cb["M1"][:], m1t[:],
                                 start=True, stop=False)
                nc.tensor.matmul(corrp[:], cb["M2P"][:], m2t[:],
                                 start=False, stop=True)
                corrsb = wk.tile([L, HE], FP, tag="corrsb", name="corrsb")
                nc.scalar.copy(corrsb[:], corrp[:])
                nc.sync.dma_start(out=co.ap()[s], in_=corrsb[:])

                # ---- transpose corr -> [128c, 96] x4 for top-k ----
                corrTp = ps128.tile([128, NT * L], FP, tag="t128", name="corrTp")
                for T in range(NT):
                    nc.tensor.transpose(
                        corrTp[:, T * L:(T + 1) * L],
                        corrsb[:, T * 128:(T + 1) * 128],
                        cb["ident"][:L, :L],
                    )
                corrT = wk.tile([128, NT * L], FP, tag="corrT", name="corrT")
                nc.scalar.copy(corrT[:], corrTp[:])

                # ---- top-k (max8 + indices) ----
                t8v = sm.tile([128, 8 * NT], FP, tag="t8v", name="t8v")
                t8i = sm.tile([128, 8 * NT], U32, tag="t8i", name="t8i")
                for T in range(NT):
                    nc.vector.max(t8v[:, T * 8:(T + 1) * 8],
                                  corrT[:, T * L:(T + 1) * L])
                    nc.vector.max_index(t8i[:, T * 8:(T + 1) * 8],
                                        t8v[:, T * 8:(T + 1) * 8],
                                        corrT[:, T * L:(T + 1) * L])

                # ---- batched softmax over the 4 taps of all 4 tiles ----
                wexp = sm.tile([128, TOPK * NT], FP, tag="wexp", name="wexp")
                wsum = sm.tile([128, NT], FP, tag="wsum", name="wsum")
                wrec = sm.tile([128, NT], FP, tag="wrec", name="wrec")
                wnrm = sm.tile([128, TOPK * NT], FP, tag="wnrm", name="wnrm")
                dF = sm.tile([128, TOPK * NT], FP, tag="dF", name="dF")
                t8v_4 = t8v[:].rearrange("p (t e) -> p t e", e=8)[:, :, 0:TOPK]
                nc.scalar.activation(
                    wexp[:].rearrange("p (t e) -> p t e", e=TOPK),
                    t8v_4, AF.Exp)
                nc.vector.tensor_reduce(
                    wsum[:], wexp[:].rearrange("p (t e) -> p t e", e=TOPK),
                    axis=mybir.AxisListType.X, op=ALU.add)
                nc.vector.reciprocal(wrec[:], wsum[:])
                wrec_ap = wrec[:]
                wrec_b = bass.AP(wrec_ap.tensor, wrec_ap.offset,
                                 [wrec_ap.ap[0], [1, NT], [0, TOPK]])
                nc.vector.tensor_mul(
                    wnrm[:].rearrange("p (t e) -> p t e", e=TOPK),
                    wexp[:].rearrange("p (t e) -> p t e", e=TOPK), wrec_b)
                t8i_4 = t8i[:].rearrange("p (t e) -> p t e", e=8)[:, :, 0:TOPK]
                nc.vector.tensor_copy(
                    dF[:].rearrange("p (t e) -> p t e", e=TOPK), t8i_4)

                # ---- weighted one-hot e[c, (T,i,s)] in bf16 ----
                est = wk.tile([128, NT * TOPK * L], BF, tag="est", name="est")
                for T in range(NT):
                    for i in range(TOPK):
                        nc.vector.tensor_scalar(
                            out=est[:, (T * TOPK + i) * L:(T * TOPK + i + 1) * L],
                            in0=cb["iotab"][:],
                            scalar1=dF[:, T * TOPK + i:T * TOPK + i + 1],
                            scalar2=wnrm[:, T * TOPK + i:T * TOPK + i + 1],
                            op0=ALU.is_equal,
                            op1=ALU.mult)

                # ---- transpose e back to [s-major, c] in 128-chunks ----
                uTsb = wk.tile([128, 3 * HE], BF, tag="uTsb", name="uTsb")
                for j in range(3):
                    uTp = ps128.tile([128, HE], BF, tag="t128", name="uTp")
                    for T in range(NT):
                        nc.tensor.transpose(
                            uTp[:, T * 128:(T + 1) * 128],
                            est[:, T * TOPK * L + j * 128:
                                   T * TOPK * L + (j + 1) * 128],
                            cb["identb"][:])
                    nc.scalar.copy(uTsb[:, j * HE:(j + 1) * HE], uTp[:])

                # ---- UF = replicated-DFT @ uT (also sums the 4 taps) ----
                UFp = ps98.tile([PW, HE], FP, tag="f98", name="UFp")
                nc.tensor.matmul(UFp[:], cb["DR0b"][:], uTsb[:, 0:HE],
                                 start=True, stop=False)
                nc.tensor.matmul(UFp[:], cb["DR1b"][:], uTsb[:, HE:2 * HE],
                                 start=False, stop=False)
                nc.tensor.matmul(UFp[:], cb["DR2b"][:], uTsb[:, 2 * HE:3 * HE],
                                 start=False, stop=True)
                ufs = wk.tile([PW, HE], BF, tag="ufs", name="ufs")
                nc.scalar.copy(ufs[:], UFp[:])

                # ---- forward DFTs of v (bf16) ----
                VFp = ps98.tile([PW, HE], FP, tag="f98", name="VFp")
                VFsp = ps98.tile([PW, HE], FP, tag="f98", name="VFsp")
                nc.tensor.matmul(VFp[:], cb["Dqb"][:], vt[:])
                nc.tensor.matmul(VFsp[:], cb["Dswb"][:], vt[:])
                vfs = wk.tile([PW, HE], BF, tag="vfs", name="vfs")
                vfss = wk.tile([PW, HE], BF, tag="vfss", name="vfss")
                nc.scalar.copy(vfs[:], VFp[:])
                nc.scalar.copy(vfss[:], VFsp[:])

                g1t = wk.tile([PW, HE], BF, tag="g1t", name="g1t")
                g2t = wk.tile([PW, HE], BF, tag="g2t", name="g2t")
                nc.vector.tensor_mul(g1t[:], ufs[:], vfs[:])
                nc.vector.tensor_mul(g2t[:], ufs[:], vfss[:])

                aggp = ps96.tile([L, HE], FP, tag="f96", name="aggp")
                nc.tensor.matmul(aggp[:], cb["M1b"][:], g1t[:],
                                 start=True, stop=False)
                nc.tensor.matmul(aggp[:], cb["M2Gb"][:], g2t[:],
                                 start=False, stop=True)
                aggsb = wk.tile([L, HE], FP, tag="aggsb", name="aggsb")
                nc.scalar.copy(aggsb[:], aggp[:])
                nc.sync.dma_start(out=vo.ap()[s], in_=aggsb[:])

    nc.compile()
    return nc, cf32, cb16


_PROG_CACHE = {}


def _get_program(n_slices):
    if n_slices not in _PROG_CACHE:
        _PROG_CACHE[n_slices] = _build_program(n_slices)
    return _PROG_CACHE[n_slices]


def _make_in_maps(q, k, v, cf32, cb16, n_slices):
    in_maps = []
    for c in range(NCORES):
        m = {"qs": q[c * n_slices:(c + 1) * n_slices],
             "ks": k[c * n_slices:(c + 1) * n_slices],
             "vs": v[c * n_slices:(c + 1) * n_slices]}
        m.update(cf32)
        m.update(cb16)
        in_maps.append(m)
    return in_maps


def _prep(queries, keys, values):
    q = np.ascontiguousarray(queries, dtype=np.float32).reshape(NSLICES, L, HE)
    k = np.ascontiguousarray(keys, dtype=np.float32).reshape(NSLICES, L, HE)
    v = np.ascontiguousarray(values, dtype=np.float32).reshape(NSLICES, L, HE)
    total = S * NCORES
    pad = total - NSLICES
    if pad:
        z = np.zeros((pad, L, HE), np.float32)
        q = np.concatenate([q, z], 0)
        k = np.concatenate([k, z], 0)
        v = np.concatenate([v, z], 0)
    return q, k, v.astype(BF_NP)


def kernel(queries, keys, values, attn_mask=None):
    q, k, v = _prep(queries, keys, values)
    nc, cf32, cb16 = _get_program(S)
    in_maps = _make_in_maps(q, k, v, cf32, cb16, S)
    res = run_bass_kernel_spmd(nc, in_maps, core_ids=list(range(NCORES)))
    corr = np.concatenate([r["co"] for r in res.results], 0)[:NSLICES]
    agg = np.concatenate([r["vo"] for r in res.results], 0)[:NSLICES]
    return (agg.reshape(B, N, L, H, E).astype(np.float32),
            corr.reshape(B, N, L, H, E).astype(np.float32))


# revision 19
# speedup vs baseline: 111.8494x; 111.8494x over previous
"""AutoCorrelation (Autoformer) Bass kernel for Trainium2, 8 NeuronCores.

Inputs (full): queries/keys/values [4, 207, 96, 8, 64] f32, attn_mask scalar.
Outputs: tuple (V, corr), each [4, 207, 96, 8, 64] f32.

Strategy: flatten (B,N) -> 828 independent slices of shape [L=96, H*E=512],
sharded over 8 cores (pad to 832 = 8*104). Each slice is processed fully
on-chip in the natural [L, HE] layout:
  - rfft/irfft of the circular cross-correlation are DFT matmuls contracting
    over L (or freq) on the partition axis; the +/- recombination of the
    complex product is absorbed into the inverse-DFT matmul weights. The
    corr path stays fp32 so top-k ranks agree with the fp32 reference.
  - top-k=4 per channel via DVE max8/max_index on the transposed corr.
  - the circular gather runs in the frequency domain (bf16): a weighted
    one-hot of the delays is built with fused tensor_scalar (is_equal, mult)
    ops, transposed on the tensor engine, DFT'd with a replicated-DFT matmul
    (which also sums the 4 taps), multiplied with VF, and inverse-DFT'd.
"""

import math
import sys

sys.path.insert(0, "/opt/trn_rl_repo")

import ml_dtypes
import numpy as np

import concourse.bacc as bacc
import concourse.bass as bass
import concourse.mybir as mybir
from concourse import tile
from concourse.bass_utils import run_bass_kernel_spmd

B, N, L, H, E = 4, 207, 96, 8, 64
HE = H * E          # 512
F = L // 2 + 1      # 49
F2 = 2 * F          # 98
PW = 128            # padded freq partitions: Re 0..48, Im 64..112
IMG = 64
TOPK = 4            # int(log(96))
NCORES = 8
NSLICES = B * N                      # 828
S = math.ceil(NSLICES / NCORES)      # 104
NT = HE // 128                       # 4 channel tiles of 128

FP = mybir.dt.float32
BF = mybir.dt.bfloat16
U32 = mybir.dt.uint32
AF = mybir.ActivationFunctionType
ALU = mybir.AluOpType
BF_NP = ml_dtypes.bfloat16


def _consts():
    t = np.arange(L)[:, None]
    f = np.arange(F)[None, :]
    C = np.cos(2 * np.pi * f * t / L)
    Sm = np.sin(2 * np.pi * f * t / L)
    Z15 = np.zeros((L, 64 - F))
    Dq = np.concatenate([C, Z15, -Sm, Z15], axis=1).astype(np.float32)   # [96, 128]
    wf = np.full(F, 2.0)
    wf[0] = 1.0
    wf[F - 1] = 1.0
    tau = np.arange(L)[None, :]
    fc = np.arange(F)[:, None]
    IC = (wf[:, None] / L) * np.cos(2 * np.pi * fc * tau / L)   # [49, 96]
    ISn = (wf[:, None] / L) * np.sin(2 * np.pi * fc * tau / L)
    Z15r = np.zeros((64 - F, L))
    M1 = np.concatenate([IC, Z15r, IC, Z15r], axis=0).astype(np.float32)   # [128, 96]
    M2P = np.concatenate([ISn, Z15r, -ISn, Z15r], axis=0).astype(np.float32)
    DR = Dq[np.arange(3 * 128) % L, :].astype(np.float32)       # [384, 128]
    iota = np.tile(np.arange(L, dtype=np.float32), (128, 1))    # [128, 96]
    ident = np.eye(128, dtype=np.float32)
    f32 = dict(Dq=Dq, M1=M1, M2P=M2P, ident=ident)
    b16 = dict(Dqb=Dq, M1b=M1, M2Pb=M2P,
               DR0b=DR[0:128], DR1b=DR[128:256], DR2b=DR[256:384],
               iotab=iota, identb=ident)
    b16 = {k: v.astype(BF_NP) for k, v in b16.items()}
    return f32, b16


def _build_program(n_slices):
    nc = bacc.Bacc("TRN2", target_bir_lowering=False, debug=False,
                   num_devices=NCORES)
    qs = nc.dram_tensor("qs", [n_slices, L, HE], FP, kind="ExternalInput")
    ks = nc.dram_tensor("ks", [n_slices, L, HE], FP, kind="ExternalInput")
    vs = nc.dram_tensor("vs", [n_slices, L, HE], BF, kind="ExternalInput")
    cf32, cb16 = _consts()
    cdram = {}
    for k, v in cf32.items():
        cdram[k] = nc.dram_tensor(k, list(v.shape), FP, kind="ExternalInput")
    for k, v in cb16.items():
        cdram[k] = nc.dram_tensor(k, list(v.shape), BF, kind="ExternalInput")
    co = nc.dram_tensor("co", [n_slices, L, HE], FP, kind="ExternalOutput")
    vo = nc.dram_tensor("vo", [n_slices, L, HE], FP, kind="ExternalOutput")

    with tile.TileContext(nc) as tc:
        with (
            tc.tile_pool(name="const", bufs=1) as cpool,
            tc.tile_pool(name="io", bufs=4) as io,
            tc.tile_pool(name="work", bufs=3) as wk,
            tc.tile_pool(name="small", bufs=4) as sm,
            tc.tile_pool(name="psA", bufs=1, space="PSUM") as psA,
            tc.tile_pool(name="psB", bufs=1, space="PSUM") as psB,
            tc.tile_pool(name="psC", bufs=1, space="PSUM") as psC,
            tc.tile_pool(name="psV", bufs=1, space="PSUM") as psV,
            tc.tile_pool(name="psUA", bufs=2, space="PSUM") as psUA,
            tc.tile_pool(name="psCT", bufs=1, space="PSUM") as psCT,
            tc.tile_pool(name="psT", bufs=1, space="PSUM") as psT,
        ):
            cb = {}
            for k, v in cf32.items():
                cb[k] = cpool.tile(list(v.shape), FP, tag=k, name=k)
                nc.sync.dma_start(out=cb[k][:], in_=cdram[k].ap())
            for k, v in cb16.items():
                cb[k] = cpool.tile(list(v.shape), BF, tag=k, name=k)
                nc.sync.dma_start(out=cb[k][:], in_=cdram[k].ap())

            for s in range(n_slices):
                qt = io.tile([L, HE], FP, tag="qt", name="qt")
                kt = io.tile([L, HE], FP, tag="kt", name="kt")
                vt = io.tile([L, HE], BF, tag="vt", name="vt")
                nc.sync.dma_start(out=qt[:], in_=qs.ap()[s])
                nc.sync.dma_start(out=kt[:], in_=ks.ap()[s])
                nc.sync.dma_start(out=vt[:], in_=vs.ap()[s])

                # ---- forward DFTs of q, k (fp32) ----
                QFp = psA.tile([PW, HE], FP, tag="QFp", name="QFp")
                KFp = psB.tile([PW, HE], FP, tag="KFp", name="KFp")
                nc.tensor.matmul(QFp[:], cb["Dq"][:], qt[:])
                nc.tensor.matmul(KFp[:], cb["Dq"][:], kt[:])

                # ---- complex products; recombination folded into M1/M2P ----
                qfs = wk.tile([PW, HE], FP, tag="qfs", name="qfs")
                nc.scalar.copy(qfs[:], QFp[:])
                m1t = wk.tile([PW, HE], FP, tag="m1t", name="m1t")
                m2t = wk.tile([PW, HE], FP, tag="m2t", name="m2t")
                nc.vector.tensor_mul(m1t[:], qfs[:], KFp[:])
                nc.vector.tensor_mul(m2t[:IMG, :], qfs[:IMG, :], KFp[IMG:, :])
                nc.vector.tensor_mul(m2t[IMG:, :], qfs[IMG:, :], KFp[:IMG, :])

                # ---- inverse DFT directly in transposed [128c, 96] form:
                # corrT chunk = (M.T @ m)[.,chunk].T = m_chunk.T @ M ----
                corrTp = psCT.tile([128, NT * L], FP, tag="corrTp", name="corrTp")
                for T in range(NT):
                    nc.tensor.matmul(corrTp[:, T * L:(T + 1) * L],
                                     m1t[:, T * 128:(T + 1) * 128],
                                     cb["M1"][:], start=True, stop=False)
                    nc.tensor.matmul(corrTp[:, T * L:(T + 1) * L],
                                     m2t[:, T * 128:(T + 1) * 128],
                                     cb["M2P"][:], start=False, stop=True)
                corrT = wk.tile([128, NT * L], FP, tag="corrT", name="corrT")
                nc.scalar.copy(corrT[:], corrTp[:])

                # ---- corr output: transpose corrT back to [96, 512] ----
                corrp = psC.tile([L, HE], FP, tag="corrp", name="corrp")
                for T in range(NT):
                    nc.tensor.transpose(
                        corrp[:, T * 128:(T + 1) * 128],
                        corrT[:, T * L:(T + 1) * L],
                        cb["ident"][:],
                    )
                corrsb = wk.tile([L, HE], FP, tag="corrsb", name="corrsb")
                nc.scalar.copy(corrsb[:], corrp[:])
                nc.sync.dma_start(out=co.ap()[s], in_=corrsb[:])

                # ---- top-k (max8 + indices) ----
                t8v = sm.tile([128, 8 * NT], FP, tag="t8v", name="t8v")
                t8i = sm.tile([128, 8 * NT], U32, tag="t8i", name="t8i")
                for T in range(NT):
                    nc.vector.max(t8v[:, T * 8:(T + 1) * 8],
                                  corrT[:, T * L:(T + 1) * L])
                    nc.vector.max_index(t8i[:, T * 8:(T + 1) * 8],
                                        t8v[:, T * 8:(T + 1) * 8],
                                        corrT[:, T * L:(T + 1) * L])

                # ---- batched softmax over the 4 taps of all 4 tiles ----
                wexp = sm.tile([128, TOPK * NT], FP, tag="wexp", name="wexp")
                wsum = sm.tile([128, NT], FP, tag="wsum", name="wsum")
                wrec = sm.tile([128, NT], FP, tag="wrec", name="wrec")
                wnrm = sm.tile([128, TOPK * NT], FP, tag="wnrm", name="wnrm")
                dF = sm.tile([128, TOPK * NT], FP, tag="dF", name="dF")
                t8v_4 = t8v[:].rearrange("p (t e) -> p t e", e=8)[:, :, 0:TOPK]
                nc.scalar.activation(
                    wexp[:].rearrange("p (t e) -> p t e", e=TOPK),
                    t8v_4, AF.Exp)
                nc.vector.tensor_reduce(
                    wsum[:], wexp[:].rearrange("p (t e) -> p t e", e=TOPK),
                    axis=mybir.AxisListType.X, op=ALU.add)
                nc.vector.reciprocal(wrec[:], wsum[:])
                wrec_ap = wrec[:]
                wrec_b = bass.AP(wrec_ap.tensor, wrec_ap.offset,
                                 [wrec_ap.ap[0], [1, NT], [0, TOPK]])
                nc.vector.tensor_mul(
                    wnrm[:].rearrange("p (t e) -> p t e", e=TOPK),
                    wexp[:].rearrange("p (t e) -> p t e", e=TOPK), wrec_b)
                t8i_4 = t8i[:].rearrange("p (t e) -> p t e", e=8)[:, :, 0:TOPK]
                nc.vector.tensor_copy(
                    dF[:].rearrange("p (t e) -> p t e", e=TOPK), t8i_4)

                # ---- weighted one-hot e[c, (T,i,s)] in bf16 ----
                est = wk.tile([128, NT * TOPK * L], BF, tag="est", name="est")
                for T in range(NT):
                    for i in range(TOPK):
                        nc.vector.tensor_scalar(
                            out=est[:, (T * TOPK + i) * L:(T * TOPK + i + 1) * L],
                            in0=cb["iotab"][:],
                            scalar1=dF[:, T * TOPK + i:T * TOPK + i + 1],
                            scalar2=wnrm[:, T * TOPK + i:T * TOPK + i + 1],
                            op0=ALU.is_equal,
                            op1=ALU.mult)

                # ---- transpose e back to [s-major, c] in 128-chunks ----
                uTsb = wk.tile([128, 3 * HE], BF, tag="uTsb", name="uTsb")
                for w in range(2):
                    uTw = psT.tile([128, 6 * 128], BF, tag="tpose", name="uTw")
                    for x in range(6):
                        j, T = divmod(w * 6 + x, NT)
                        nc.tensor.transpose(
                            uTw[:, x * 128:(x + 1) * 128],
                            est[:, T * TOPK * L + j * 128:
                                   T * TOPK * L + (j + 1) * 128],
                            cb["identb"][:])
                    nc.scalar.copy(uTsb[:, w * 768:(w + 1) * 768], uTw[:])

                # ---- UF = replicated-DFT @ uT (also sums the 4 taps) ----
                UFp = psUA.tile([PW, HE], FP, tag="UFagg", name="UFp")
                nc.tensor.matmul(UFp[:], cb["DR0b"][:], uTsb[:, 0:HE],
                                 start=True, stop=False)
                nc.tensor.matmul(UFp[:], cb["DR1b"][:], uTsb[:, HE:2 * HE],
                                 start=False, stop=False)
                nc.tensor.matmul(UFp[:], cb["DR2b"][:], uTsb[:, 2 * HE:3 * HE],
                                 start=False, stop=True)
                # swapped-half copy of UF ([Ui;Ur]) so g2 is base-aligned
                ufsw = wk.tile([PW, HE], BF, tag="ufsw", name="ufsw")
                nc.scalar.copy(ufsw[:IMG, :], UFp[IMG:, :])
                nc.scalar.copy(ufsw[IMG:, :], UFp[:IMG, :])

                # ---- forward DFTs of v (bf16) ----
                VFp = psV.tile([PW, HE], FP, tag="VFp", name="VFp")
                nc.tensor.matmul(VFp[:], cb["Dqb"][:], vt[:])
                vfs = wk.tile([PW, HE], BF, tag="vfs", name="vfs")
                nc.scalar.copy(vfs[:], VFp[:])
                g1t = wk.tile([PW, HE], BF, tag="g1t", name="g1t")
                g2t = wk.tile([PW, HE], BF, tag="g2t", name="g2t")
                nc.vector.tensor_mul(g1t[:], vfs[:], UFp[:])
                nc.vector.tensor_mul(g2t[:], ufsw[:], vfs[:])

                aggp = psUA.tile([PW, HE], FP, tag="UFagg", name="aggp")
                nc.tensor.matmul(aggp[:L, :], cb["M1b"][:], g1t[:],
                                 start=True, stop=False)
                nc.tensor.matmul(aggp[:L, :], cb["M2Pb"][:], g2t[:],
                                 start=False, stop=True)
                aggsb = wk.tile([L, HE], FP, tag="aggsb", name="aggsb")
                nc.scalar.copy(aggsb[:], aggp[:L, :])
                nc.sync.dma_start(out=vo.ap()[s], in_=aggsb[:])

    nc.compile()
    return nc, cf32, cb16


_PROG_CACHE = {}


def _get_program(n_slices):
    if n_slices not in _PROG_CACHE:
        _PROG_CACHE[n_slices] = _build_program(n_slices)
    return _PROG_CACHE[n_slices]


def _make_in_maps(q, k, v, cf32, cb16, n_slices):
    in_maps = []
    for c in range(NCORES):
        m = {"qs": q[c * n_slices:(c + 1) * n_slices],
             "ks": k[c * n_slices:(c + 1) * n_slices],
             "vs": v[c * n_slices:(c + 1) * n_slices]}
        m.update(cf32)
        m.update(cb16)
        in_maps.append(m)
    return in_maps


def _prep(queries, keys, values):
    q = np.ascontiguousarray(queries, dtype=np.float32).reshape(NSLICES, L, HE)
    k = np.ascontiguousarray(keys, dtype=np.float32).reshape(NSLICES, L, HE)
    v = np.ascontiguousarray(values, dtype=np.float32).reshape(NSLICES, L, HE)
    total = S * NCORES
    pad = total - NSLICES
    if pad:
        z = np.zeros((pad, L, HE), np.float32)
        q = np.concatenate([q, z], 0)
        k = np.concatenate([k, z], 0)
        v = np.concatenate([v, z], 0)
    return q, k, v.astype(BF_NP)


def kernel(queries, keys, values, attn_mask=None):
    q, k, v = _prep(queries, keys, values)
    nc, cf32, cb16 = _get_program(S)
    in_maps = _make_in_maps(q, k, v, cf32, cb16, S)
    res = run_bass_kernel_spmd(nc, in_maps, core_ids=list(range(NCORES)))
    corr = np.concatenate([r["co"] for r in res.results], 0)[:NSLICES]
    agg = np.concatenate([r["vo"] for r in res.results], 0)[:NSLICES]
    return (agg.reshape(B, N, L, H, E).astype(np.float32),
            corr.reshape(B, N, L, H, E).astype(np.float32))
